# revision 111
# baseline (speedup 1.0000x reference)
"""KoLeo loss kernel for 8 trn2 NeuronCores — fp8 DoubleRow, rebalanced.

Math: L2-normalize rows of X [16384,768]; for each row find the nearest
neighbor (self excluded) by cosine similarity; loss =
-mean(log(||xn_i - xn_NN(i)||)).  Rows are unit vectors, so
||xn_i - xn_j||^2 = 2 - 2*<xn_i, xn_j> and only the per-row MAX inner
product is needed on device.  Device returns per-query chunk-reduced
stats (BV = exact max over the DVE-drained columns, AS = exp-sum over
the ACT-drained columns); the host finishes with
bestv = max(BV, ln(AS)/GAMMA + EXPB), LI = ln(2 - bestv/128),
loss = -0.5 * mean(LI).

Sharding: each core c receives X ROTATED by -2048*c rows, so every core
sees its own 2048 queries as rows [0:2048) ("chunk 0") and the full key
set as chunks 0..7.  This keeps the SPMD module fully static (no
per-core offsets), lets the queries reuse chunk 0's staged fp8
transpose (no separate Q pipeline), and makes the self-mask input
identical on every core.

Resource budget (TimelineSim cost model): PE fp8 DoubleRow matmuls
~167us; the n^2/8 similarity values must each cross ACT or DVE (GPSIMD
has no PSUM port); DMA engines are a shared ~360GB/s pipe; HWDGE/SWDGE
sequencer issue time is significant.  Design choices:
  - X loaded via batched gpsimd SWDGE casting DMAs (fp32 DRAM -> bf16
    SBUF, 8 row tiles per DMA): halves the dominant X DMA-engine time
    and the SBUF footprint, and bf16 operands give DVE 2x throughput
    on the square pass.
  - ACT: ONLY exp-logsumexp drains (whole [128,2048] psum tile per
    instruction, accum_out -> AC).
  - DVE: exact reduce_max drains (every DVE_EVERY-th (m,kc) -> BM),
    fused square+accum (scalar_tensor_tensor, bf16 2x), rsqrt Newton
    chain, QTD repack, and a small share of fp8 scales.
  - Pool: SWDGE load issue + most fp8 scale-copies.
  - PSUM phase 1 (kc 0-4): per (m,kc) TWO tiles, psA [128,1536] (ACT
    exp drain) and psB [128,512] (DVE exact max), double-buffered = 8
    banks.  Separate tiles, not slices of one: two readers of one psum
    tile get a read-after-read sync edge from the dependency annotator,
    serializing the drains.  Phase 2 (kc 5-7, staging mostly done):
    re-pooled to [128,1024] x2 + [128,1024] x2 so DVE takes half of
    every drain.  The self-containing 512-block always routes to psB
    (the mask matmul carries psB's stop; the ACT exp racing the late
    self-mask was a real-HW corruption mode).  Staging for later chunks
    is emitted in large up-front bursts; matmuls issue cc-outer.

Per core:
  1. Normalize: gpsimd cast-DMA loads (bf16); DVE fused square+accum
     (prologue alternates with ACT Square); rsqrt on DVE via the Quake
     bit hack + 2 Newton steps ([128,16] batches; the x16 fp8 scale is
     folded into the last Newton step); scale-copy to fp8 rows -> DRAM.
  2. Transpose via XBAR DMA (fp8 row pairs as uint16) into per-chunk
     KTC [128 k-pair, 3*2048] tiles; QTD stationary repacked from
     chunk 0's KTC (queries == chunk 0).
  3. Sweep: per (kc, m): one [128,2048] PSUM tile = 4x3 fp8 DoubleRow
     matmuls (256-deep) + a 128-wide matmul adding I8^T @ MASKS[kc]
     = -1024*I on the self block (MASKS: -128*I at kc=0, else 0) —
     exact self masking, carrying the tile's last stop.  Drain: every
     psB block -> DVE reduce_max -> BM; psA -> ACT fused
     exp(GAMMA*(dots-EXPB)) + accumulate -> AC (logsumexp ~ max).
  4. Finish: device reduces BM/AC over chunks -> BV/AS -> DRAM; the
     scalar tail (ln, max, log-distance, mean) runs on the host.
"""

import os

import ml_dtypes
import numpy as np

import concourse.bacc as bacc
import concourse.mybir as mybir
import concourse.tile as tile
from concourse.bass_utils import run_bass_kernel_spmd

F32 = mybir.dt.float32
BF16 = mybir.dt.bfloat16
FP8 = mybir.dt.float8e4
U16 = mybir.dt.uint16
U32 = mybir.dt.uint32

N = 16384
D = 768
NCORES = 8
QPC = N // NCORES          # 2048 queries per core
MT = QPC // 128            # 16 query tiles per core
NKC = N // 2048            # 8 key chunks of 2048
NCC = 3                    # k-pair blocks (768 = 3 * 256)
SB = 16                    # rsqrt batching (row tiles per batch)
LB = 8                     # row tiles per casting load DMA
GAMMA = 1.5                # logsumexp sharpness on the 256*s scale
EXPB = 35.0                # exp offset (scaled units) keeping Ln in range
# drain split at a PSUM-tile boundary: ACT drains psA = keys [0:1536) via
# exp-logsumexp, DVE drains psB = keys [1536:2048) via exact reduce_max.
# Separate tiles, not slices of one tile: two readers of one psum tile get
# a read-after-read sync edge from the dependency annotator, serializing
# the ACT and DVE drains.
WA = 1536
POOL_SCALE_MOD = 5         # every 8th steady-state scale goes to DVE

LAST_EXEC_NS = None


def _build_nc():
    nc = bacc.Bacc("TRN2")

    X = nc.dram_tensor("X", [N, D], F32, kind="ExternalInput")
    I8 = nc.dram_tensor("I8", [128, 128], FP8, kind="ExternalInput")
    MASKS = nc.dram_tensor("MASKS", [NKC * 128, 128], FP8, kind="ExternalInput")
    BV = nc.dram_tensor("BV", [128, MT], F32, kind="ExternalOutput")
    AS = nc.dram_tensor("AS", [128, MT], F32, kind="ExternalOutput")

    with tile.TileContext(nc) as tc:
        with (
            tc.tile_pool(name="dram", bufs=3, space="DRAM") as dpool,
            tc.tile_pool(name="xt", bufs=5) as xpool,
            tc.tile_pool(name="sq", bufs=2) as sqpool,
            tc.tile_pool(name="x8", bufs=3) as x8pool,
            tc.tile_pool(name="stat", bufs=8) as stat,
            tc.tile_pool(name="keys", bufs=3) as kpool,
            tc.tile_pool(name="esc", bufs=2) as escpool,
            tc.tile_pool(name="persist", bufs=1) as persist,
            tc.tile_pool(name="accp", bufs=1) as accp,
            tc.tile_pool(name="bmp", bufs=1) as bmp,
        ):
            ident8 = persist.tile([128, 128], FP8, tag="i8")
            nc.sync.dma_start(ident8, I8[:, :])
            masks = persist.tile([128, NKC * 128], FP8, tag="masks")
            for kc in range(NKC):
                nc.sync.dma_start(
                    masks[:, kc * 128:(kc + 1) * 128],
                    MASKS[kc * 128:(kc + 1) * 128, :],
                )
            # per-(m,kc) best values (DVE path) and exp sums (ACT path);
            # separate pools so their writes never false-serialize
            BM = bmp.tile([128, MT * NKC], F32, tag="bm")
            nc.vector.memset(BM, -1e30)
            AC = accp.tile([128, MT * NKC], F32, tag="ac")
            nc.vector.memset(AC, 0.0)
            ebias = persist.tile([128, 1], F32, tag="ebias")
            nc.vector.memset(ebias, -GAMMA * EXPB)
            # contiguous DoubleRow stationary layout: [cc][m][ksub 2][j 128]
            QTD = persist.tile([128, NCC * MT * 256], FP8, tag="qtd")

            def stage_load_gen(src_row0, nrt, sink, prologue=False,
                               lb=LB, sbatch=SB):
                """Generator: load+square nrt row tiles, yielding after
                each tile so the caller can interleave with sweep work.
                Appends (xt_slice, rs_ap) pairs to sink
                ([128,1] rs = 16/||row||).

                Loads: one gpsimd SWDGE casting DMA (f32 DRAM -> bf16
                SBUF) per LB row tiles.  Squares: DVE fused
                scalar_tensor_tensor x*x with accum_out (bf16 2x mode);
                prologue even tiles use ACT Square+accum instead.
                rsqrt: DVE Quake initial guess + 2 Newton steps on
                [128, SB] batches; the last Newton step folds in the
                x16 fp8 scale.
                """
                for b0 in range(0, nrt, sbatch):
                    nb = min(sbatch, nrt - b0)
                    n2c = stat.tile([128, SB], F32, tag="n2c")
                    xts = []
                    for l0 in range(0, nb, lb):
                        nl = min(lb, nb - l0)
                        row0 = src_row0 + (b0 + l0) * 128
                        xt = xpool.tile([128, LB * D], BF16, tag="xt")
                        nc.gpsimd.dma_start(
                            xt[:, 0:nl * D].rearrange(
                                "p (t d) -> p t d", t=nl),
                            X[row0:row0 + nl * 128, :].rearrange(
                                "(t p) d -> p t d", p=128),
                        )
                        for t in range(nl):
                            xs = xt[:, t * D:(t + 1) * D]
                            ti = l0 + t
                            sq = sqpool.tile([128, D], BF16, tag="sq")
                            if prologue and ti % 2 == 1:
                                nc.scalar.activation(
                                    sq, xs,
                                    mybir.ActivationFunctionType.Square,
                                    accum_out=n2c[:, ti:ti + 1])
                            else:
                                nc.vector.scalar_tensor_tensor(
                                    out=sq, in0=xs, scalar=1.0, in1=xs,
                                    op0=mybir.AluOpType.mult,
                                    op1=mybir.AluOpType.mult,
                                    accum_out=n2c[:, ti:ti + 1])
                            xts.append(xs)
                            yield
                    nn = n2c[:, 0:nb]
                    t1 = stat.tile([128, SB], U32, tag="t1")
                    nc.vector.tensor_scalar(
                        t1[:, 0:nb], nn.bitcast(U32), 1, None,
                        op0=mybir.AluOpType.logical_shift_right)
                    t2 = stat.tile([128, SB], U32, tag="t2")
                    # 0x5f3759df - t1 == ~(t1 + ~0x5f3759df); the add stays
                    # below 2^32 (the DVE uint add saturates, not wraps)
                    nc.vector.tensor_scalar(
                        t2[:, 0:nb], t1[:, 0:nb], 0xA0C8A620, None,
                        op0=mybir.AluOpType.add)
                    nc.vector.tensor_scalar(
                        t2[:, 0:nb], t2[:, 0:nb], 0xFFFFFFFF, None,
                        op0=mybir.AluOpType.bitwise_xor)
                    y0 = t2.bitcast(F32)[:, 0:nb]
                    aa = stat.tile([128, SB], F32, tag="aa")
                    nc.vector.tensor_tensor(
                        out=aa[:, 0:nb], in0=y0, in1=y0,
                        op=mybir.AluOpType.mult)
                    nc.vector.tensor_tensor(
                        out=aa[:, 0:nb], in0=aa[:, 0:nb], in1=nn,
                        op=mybir.AluOpType.mult)
                    nc.vector.tensor_scalar(
                        aa[:, 0:nb], aa[:, 0:nb], -0.5, 1.5,
                        op0=mybir.AluOpType.mult, op1=mybir.AluOpType.add)
                    y1 = stat.tile([128, SB], F32, tag="y1")
                    nc.vector.tensor_tensor(
                        out=y1[:, 0:nb], in0=y0, in1=aa[:, 0:nb],
                        op=mybir.AluOpType.mult)
                    bb = stat.tile([128, SB], F32, tag="bb")
                    nc.vector.tensor_tensor(
                        out=bb[:, 0:nb], in0=y1[:, 0:nb], in1=y1[:, 0:nb],
                        op=mybir.AluOpType.mult)
                    nc.vector.tensor_tensor(
                        out=bb[:, 0:nb], in0=bb[:, 0:nb], in1=nn,
                        op=mybir.AluOpType.mult)
                    # fold the x16 fp8 scale into the last Newton step
                    nc.vector.tensor_scalar(
                        bb[:, 0:nb], bb[:, 0:nb], -8.0, 24.0,
                        op0=mybir.AluOpType.mult, op1=mybir.AluOpType.add)
                    rsc = stat.tile([128, SB], F32, tag="rsc")
                    nc.vector.tensor_tensor(
                        out=rsc[:, 0:nb], in0=y1[:, 0:nb], in1=bb[:, 0:nb],
                        op=mybir.AluOpType.mult)
                    for t in range(nb):
                        sink.append((xts[t], rsc[:, t:t + 1]))
                    yield

            def finish_gen(pairs, dst_rows, ktc, prologue=False, ntr=1):
                """Generator: scale each staged tile into a 4-tile fp8
                batch buffer; one batched store DMA per SBB tiles (issued
                one batch late so it never waits at the SP queue head);
                then the XBAR transposes into ktc (on the ACT HWDGE to
                keep the SP queue free); yields between tiles."""
                SBB = 4
                nrows = len(pairs) * 128
                su = dst_rows.bitcast(U16)
                hr = nrows // ntr

                def xbar(h):
                    for cc in range(NCC):
                        nc.sync.dma_start_transpose(
                            ktc[:, cc * nrows + h * hr:
                                cc * nrows + (h + 1) * hr],
                            su[h * hr:(h + 1) * hr, cc * 128:(cc + 1) * 128],
                        )

                x8 = None
                pending = None  # (x8_tile, row0, nrows)
                done_rows = 0
                next_h = 0
                for t, (xs, rs) in enumerate(pairs):
                    if t % SBB == 0:
                        if pending is not None:
                            bx8, row0, nr = pending
                            nc.sync.dma_start(
                                dst_rows[row0:row0 + nr * 128, :].rearrange(
                                    "(t p) d -> p t d", p=128),
                                bx8[:, 0:nr * D].rearrange(
                                    "p (t d) -> p t d", t=nr),
                            )
                            done_rows = row0 + nr * 128
                            # fire a transpose slice as soon as its source
                            # rows are all stored
                            while (next_h < ntr - 1
                                   and (next_h + 1) * hr <= done_rows):
                                xbar(next_h)
                                next_h += 1
                        x8 = x8pool.tile([128, SBB * D], FP8, tag="x8")
                        pending = (x8, t * 128, min(SBB, len(pairs) - t))
                    if prologue:
                        eng = nc.vector if t % 2 == 1 else nc.gpsimd
                    else:
                        eng = nc.vector if t % POOL_SCALE_MOD == (
                            POOL_SCALE_MOD - 1) else nc.gpsimd
                    eng.tensor_scalar(
                        x8[:, (t % SBB) * D:(t % SBB + 1) * D], xs, rs,
                        None, op0=mybir.AluOpType.mult)
                    yield
                if pending is not None:
                    bx8, row0, nr = pending
                    nc.sync.dma_start(
                        dst_rows[row0:row0 + nr * 128, :].rearrange(
                            "(t p) d -> p t d", p=128),
                        bx8[:, 0:nr * D].rearrange("p (t d) -> p t d", t=nr),
                    )
                yield
                for h in range(next_h, ntr):
                    xbar(h)
                yield

            def run_all(gen):
                for _ in gen:
                    pass

            def sweep_chunk(kc, ktc, psA_pool, psB_pool, wa, etag,
                            feeder=None):
                kf = ktc.bitcast(FP8)  # [128, NCC*2048*2]
                nb_blocks_a = wa // 512
                for m in range(MT):
                    # two PSUM tiles per (m,kc): psA for the ACT drain,
                    # psB for the DVE drain
                    psA = psA_pool.tile([128, wa], F32, tag="psA")
                    psB = psB_pool.tile([128, 2048 - wa], F32, tag="psB")
                    # psB (the DVE exact-max tile) holds the 512-block that
                    # contains the self columns for kc=0, so the masked
                    # block is never read by the ACT exp (the mask matmul
                    # carries psB's stop, exactly the baseline structure);
                    # the remaining three blocks fill psA in order.
                    if nb_blocks_a == 3:
                        psb_set = ((m // 4),) if kc == 0 else (3,)
                    else:
                        psb_set = (2, 3)
                    tiles = []
                    pa = 0
                    pb = 0
                    for blk in range(4):
                        if blk in psb_set:
                            tiles.append((psB, pb * 512))
                            pb += 1
                        else:
                            tiles.append((psA, pa * 512))
                            pa += 1
                    # cc-outer issue order: all cc=0 matmuls first, etc.,
                    # so a chunk whose per-cc XBAR slices land staggered
                    # unblocks matmuls as early as possible (PE executes
                    # its queue in order)
                    for cc in range(NCC):
                        lhsT = QTD[
                            :, (cc * MT + m) * 256:(cc * MT + m + 1) * 256
                        ].rearrange("p (two j) -> p two j", two=2)
                        for blk in range(4):
                            ps, plo = tiles[blk]
                            needs_mask = (kc == 0 and blk == psb_set[0])
                            j0 = cc * 4096 + blk * 1024
                            rhs = kf[:, j0: j0 + 1024].rearrange(
                                "p (j two) -> p two j", two=2)
                            nc.tensor.matmul(
                                ps[:, plo:plo + 512], lhsT, rhs,
                                start=(cc == 0),
                                stop=(cc == NCC - 1 and not needs_mask),
                                perf_mode=mybir.MatmulPerfMode.DoubleRow,
                            )
                    if kc == 0:
                        # exact self-mask (chunk 0 = own rows): adds
                        # 8 * -128 * I on the self block; carries psB's
                        # stop so its reader orders after it.
                        ps, plo = tiles[psb_set[0]]
                        o = plo + (m % 4) * 128
                        nc.tensor.matmul(
                            ps[:, o:o + 128],
                            ident8,
                            masks[:, 0:128],
                            start=False, stop=True,
                            skip_group_check=True,
                        )
                    col = m * NKC + kc
                    # exact max drain of psB on DVE
                    nc.vector.reduce_max(
                        BM[:, col:col + 1], psB[:, :],
                        axis=mybir.AxisListType.X,
                    )
                    # logsumexp drain of psA on ACT:
                    # AC = sum(exp(GAMMA*(dots-EXPB)))
                    esc = escpool.tile([128, wa], BF16, tag=etag)
                    nc.scalar.activation(
                        esc, psA[:, :],
                        mybir.ActivationFunctionType.Exp,
                        scale=GAMMA, bias=ebias[:, 0:1],
                        accum_out=AC[:, col:col + 1])
                    # interleave a few staging items for future chunks so
                    # their engine work slots between this sweep's drains
                    if feeder is not None:
                        nfeed = 16 if kc == 0 else 40
                        for _ in range(nfeed):
                            if next(feeder, "done") == "done":
                                feeder = None
                                break
                if feeder is not None:
                    run_all(feeder)

            # ---- prologue: stage chunk 0 only (it doubles as the query
            # set; rows are pre-rotated per core on the host), with chunk
            # 1's loads interleaved; the first sweep needs just chunk 0 ----
            import itertools
            p0 = []
            p1 = []
            g0 = stage_load_gen(0, 16, p0, prologue=True)
            for _ in g0:
                pass
            X8C0 = dpool.tile([2048, D], FP8, tag="x8c")
            ktc0 = kpool.tile([128, NCC * 2048], U16, tag="ktc")
            for _ in itertools.zip_longest(
                finish_gen(p0, X8C0, ktc0, prologue=True),
                stage_load_gen(2048, 16, p1, prologue=True),
            ):
                pass
            # repack queries (= chunk 0) from its KTC into the contiguous
            # DoubleRow stationary layout; per-4-m groups so the first
            # sweep's stationary slices are ready before the whole repack
            for mg in range(0, MT, 4):
                for cc in range(NCC):
                    nc.vector.tensor_scalar(
                        QTD[:, (cc * MT + mg) * 256:
                            (cc * MT + mg + 4) * 256].rearrange(
                            "p (m two j) -> p m two j", m=4, two=2),
                        ktc0.bitcast(FP8)[
                            :, (cc * QPC + mg * 128) * 2:
                            (cc * QPC + (mg + 4) * 128) * 2
                        ].rearrange("p (m j two) -> p m two j", m=4, two=2),
                        1.0, None, op0=mybir.AluOpType.mult)

            # ---- staging + sweep: chunk kc sweeps while the feeder
            # interleaves finish(kc+1) then loads(kc+2) at m granularity.
            # PSUM pools are phase-split: kc 0-5 run WA=1536 with 2+2 tiles
            # (DVE is staging-loaded, so it gets the small drain); kc 6-7
            # re-pool to WA=1024 with psA x3 / psB x1 (staging done -> DVE
            # takes half the drain and 3-deep psA hides the ACT handoff) ----
            pend = {1: p1}
            ktcs = {0: ktc0}

            def make_feeder(kc):
                gens = []
                if kc + 1 < NKC:
                    X8C = dpool.tile([2048, D], FP8, tag="x8c")
                    ktcs[kc + 1] = kpool.tile(
                        [128, NCC * 2048], U16, name="ktcn", tag="ktc")
                    gens.append(finish_gen(
                        pend.pop(kc + 1), X8C, ktcs[kc + 1]))
                if kc + 2 < NKC:
                    pend[kc + 2] = []
                    gens.append(stage_load_gen(
                        (kc + 2) * 2048, 16, pend[kc + 2]))
                if len(gens) == 2:
                    # round-robin the finish and load items so scales and
                    # squares spread across the sweep
                    def _rr(a, b):
                        for x, y in itertools.zip_longest(a, b):
                            yield x
                            yield y
                    return _rr(*gens)
                if gens:
                    return itertools.chain(*gens)
                return None

            with (
                tc.tile_pool(name="psA1", bufs=2, space="PSUM") as pa1,
                tc.tile_pool(name="psB1", bufs=2, space="PSUM") as pb1,
            ):
                for kc in range(5):
                    sweep_chunk(kc, ktcs.pop(kc), pa1, pb1, 1536, "esc",
                                make_feeder(kc))
            with (
                tc.tile_pool(name="psA2", bufs=2, space="PSUM") as pa2,
                tc.tile_pool(name="psB2", bufs=2, space="PSUM") as pb2,
            ):
                for kc in range(5, NKC):
                    sweep_chunk(kc, ktcs.pop(kc), pa2, pb2, 1024, "esc2",
                                make_feeder(kc))

            # ---- finish: device only reduces per-chunk stats; the
            # scalar tail (ln, max, log-distance) runs on the host ----
            BV1 = persist.tile([128, MT], F32, tag="bv1")
            ACS = persist.tile([128, MT], F32, tag="acs")
            # two m-halves: the first half's inputs are complete ~8
            # iterations before the sweep ends, so its reduce+store can
            # overlap the tail drains
            for h0 in (0, MT // 2):
                hm = MT // 2
                nc.vector.reduce_max(
                    BV1[:, h0:h0 + hm],
                    BM[:, h0 * NKC:(h0 + hm) * NKC].rearrange(
                        "p (m k) -> p m k", k=NKC),
                    axis=mybir.AxisListType.X)
                nc.vector.reduce_sum(
                    ACS[:, h0:h0 + hm],
                    AC[:, h0 * NKC:(h0 + hm) * NKC].rearrange(
                        "p (m k) -> p m k", k=NKC),
                    axis=mybir.AxisListType.X)
                nc.sync.dma_start(BV[:, h0:h0 + hm], BV1[:, h0:h0 + hm])
                nc.sync.dma_start(AS[:, h0:h0 + hm], ACS[:, h0:h0 + hm])
    nc.compile()
    return nc


_CACHED = {}


def kernel(X: np.ndarray) -> np.ndarray:
    global LAST_EXEC_NS
    X = np.ascontiguousarray(np.asarray(X, dtype=np.float32))
    assert X.shape == (N, D)

    if "nc" not in _CACHED:
        _CACHED["nc"] = _build_nc()
    nc = _CACHED["nc"]

    eye8 = (np.eye(128) * 8.0).astype(ml_dtypes.float8_e4m3)
    # self-mask sits at chunk 0 on every core (rows are pre-rotated)
    msk = np.zeros((NKC * 128, 128), dtype=ml_dtypes.float8_e4m3)
    msk[0:128, :] = (np.eye(128) * -128.0).astype(ml_dtypes.float8_e4m3)
    in_maps = []
    for c in range(NCORES):
        in_maps.append({
            "X": np.ascontiguousarray(np.roll(X, -c * QPC, axis=0)),
            "I8": eye8,
            "MASKS": msk,
        })

    trace = os.environ.get("KOLEO_TRACE", "0") == "1"
    res = run_bass_kernel_spmd(
        nc, in_maps, core_ids=list(range(NCORES)), trace=trace,
    )
    LAST_EXEC_NS = res.exec_time_ns

    bv1 = np.concatenate(
        [r["BV"].reshape(128, MT) for r in res.results], axis=1)
    acs = np.concatenate(
        [r["AS"].reshape(128, MT) for r in res.results], axis=1)
    bv2 = np.log(np.maximum(acs, 1e-300)) / GAMMA + EXPB
    bestv = np.maximum(bv1.astype(np.float64), bv2)
    li = np.log(2.0 - bestv / 128.0)
    loss = -np.float32(0.5 * np.mean(li))
    return np.asarray(loss, dtype=np.float32)


if __name__ == "__main__":
    Xt = np.random.randn(N, D).astype(np.float32)
    print(kernel(Xt))


# revision 114
# speedup vs baseline: 1.0034x; 1.0034x over previous
"""KoLeo loss kernel for 8 trn2 NeuronCores — fp8 DoubleRow, rebalanced.

Math: L2-normalize rows of X [16384,768]; for each row find the nearest
neighbor (self excluded) by cosine similarity; loss =
-mean(log(||xn_i - xn_NN(i)||)).  Rows are unit vectors, so
||xn_i - xn_j||^2 = 2 - 2*<xn_i, xn_j> and only the per-row MAX inner
product is needed on device.  Device returns per-query chunk-reduced
stats (BV = exact max over the DVE-drained columns, AS = exp-sum over
the ACT-drained columns); the host finishes with
bestv = max(BV, ln(AS)/GAMMA + EXPB), LI = ln(2 - bestv/128),
loss = -0.5 * mean(LI).

Sharding: each core c receives X ROTATED by -2048*c rows, so every core
sees its own 2048 queries as rows [0:2048) ("chunk 0") and the full key
set as chunks 0..7.  This keeps the SPMD module fully static (no
per-core offsets), lets the queries reuse chunk 0's staged fp8
transpose (no separate Q pipeline), and makes the self-mask input
identical on every core.

Resource budget (TimelineSim cost model): PE fp8 DoubleRow matmuls
~167us; the n^2/8 similarity values must each cross ACT or DVE (GPSIMD
has no PSUM port); DMA engines are a shared ~360GB/s pipe; HWDGE/SWDGE
sequencer issue time is significant.  Design choices:
  - X loaded via batched gpsimd SWDGE casting DMAs (fp32 DRAM -> bf16
    SBUF, 8 row tiles per DMA): halves the dominant X DMA-engine time
    and the SBUF footprint, and bf16 operands give DVE 2x throughput
    on the square pass.
  - ACT: ONLY exp-logsumexp drains (whole [128,2048] psum tile per
    instruction, accum_out -> AC).
  - DVE: exact reduce_max drains (every DVE_EVERY-th (m,kc) -> BM),
    fused square+accum (scalar_tensor_tensor, bf16 2x), rsqrt Newton
    chain, QTD repack, and a small share of fp8 scales.
  - Pool: SWDGE load issue + most fp8 scale-copies.
  - PSUM phase 1 (kc 0-4): per (m,kc) TWO tiles, psA [128,1536] (ACT
    exp drain) and psB [128,512] (DVE exact max), double-buffered = 8
    banks.  Separate tiles, not slices of one: two readers of one psum
    tile get a read-after-read sync edge from the dependency annotator,
    serializing the drains.  Phase 2 (kc 5-7, staging mostly done):
    re-pooled to [128,1024] x2 + [128,1024] x2 so DVE takes half of
    every drain.  The self-containing 512-block always routes to psB
    (the mask matmul carries psB's stop; the ACT exp racing the late
    self-mask was a real-HW corruption mode).  Staging for later chunks
    is emitted in large up-front bursts; matmuls issue cc-outer.

Per core:
  1. Normalize: gpsimd cast-DMA loads (bf16); DVE fused square+accum
     (prologue alternates with ACT Square); rsqrt on DVE via the Quake
     bit hack + 2 Newton steps ([128,16] batches; the x16 fp8 scale is
     folded into the last Newton step); scale-copy to fp8 rows -> DRAM.
  2. Transpose via XBAR DMA (fp8 row pairs as uint16) into per-chunk
     KTC [128 k-pair, 3*2048] tiles; QTD stationary repacked from
     chunk 0's KTC (queries == chunk 0).
  3. Sweep: per (kc, m): one [128,2048] PSUM tile = 4x3 fp8 DoubleRow
     matmuls (256-deep) + a 128-wide matmul adding I8^T @ MASKS[kc]
     = -1024*I on the self block (MASKS: -128*I at kc=0, else 0) —
     exact self masking, carrying the tile's last stop.  Drain: every
     psB block -> DVE reduce_max -> BM; psA -> ACT fused
     exp(GAMMA*(dots-EXPB)) + accumulate -> AC (logsumexp ~ max).
  4. Finish: device reduces BM/AC over chunks -> BV/AS -> DRAM; the
     scalar tail (ln, max, log-distance, mean) runs on the host.
"""

import os

import ml_dtypes
import numpy as np

import concourse.bacc as bacc
import concourse.mybir as mybir
import concourse.tile as tile
from concourse.bass_utils import run_bass_kernel_spmd

F32 = mybir.dt.float32
BF16 = mybir.dt.bfloat16
FP8 = mybir.dt.float8e4
U16 = mybir.dt.uint16
U32 = mybir.dt.uint32

N = 16384
D = 768
NCORES = 8
QPC = N // NCORES          # 2048 queries per core
MT = QPC // 128            # 16 query tiles per core
NKC = N // 2048            # 8 key chunks of 2048
NCC = 3                    # k-pair blocks (768 = 3 * 256)
SB = 16                    # rsqrt batching (row tiles per batch)
LB = 8                     # row tiles per casting load DMA
GAMMA = 1.5                # logsumexp sharpness on the 256*s scale
EXPB = 35.0                # exp offset (scaled units) keeping Ln in range
# drain split at a PSUM-tile boundary: ACT drains psA = keys [0:1536) via
# exp-logsumexp, DVE drains psB = keys [1536:2048) via exact reduce_max.
# Separate tiles, not slices of one tile: two readers of one psum tile get
# a read-after-read sync edge from the dependency annotator, serializing
# the ACT and DVE drains.
WA = 1536
POOL_SCALE_MOD = 5         # every 8th steady-state scale goes to DVE

LAST_EXEC_NS = None


def _build_nc():
    nc = bacc.Bacc("TRN2")

    X = nc.dram_tensor("X", [N, D], F32, kind="ExternalInput")
    I8 = nc.dram_tensor("I8", [128, 128], FP8, kind="ExternalInput")
    MASKS = nc.dram_tensor("MASKS", [NKC * 128, 128], FP8, kind="ExternalInput")
    BV = nc.dram_tensor("BV", [128, MT], F32, kind="ExternalOutput")
    AS = nc.dram_tensor("AS", [128, MT], F32, kind="ExternalOutput")

    with tile.TileContext(nc) as tc:
        with (
            tc.tile_pool(name="dram", bufs=3, space="DRAM") as dpool,
            tc.tile_pool(name="xt", bufs=5) as xpool,
            tc.tile_pool(name="sq", bufs=4) as sqpool,
            tc.tile_pool(name="x8", bufs=3) as x8pool,
            tc.tile_pool(name="stat", bufs=8) as stat,
            tc.tile_pool(name="keys", bufs=3) as kpool,
            tc.tile_pool(name="esc", bufs=2) as escpool,
            tc.tile_pool(name="persist", bufs=1) as persist,
            tc.tile_pool(name="accp", bufs=1) as accp,
            tc.tile_pool(name="bmp", bufs=1) as bmp,
        ):
            ident8 = persist.tile([128, 128], FP8, tag="i8")
            nc.sync.dma_start(ident8, I8[:, :])
            masks = persist.tile([128, NKC * 128], FP8, tag="masks")
            for kc in range(NKC):
                nc.sync.dma_start(
                    masks[:, kc * 128:(kc + 1) * 128],
                    MASKS[kc * 128:(kc + 1) * 128, :],
                )
            # per-(m,kc) best values (DVE path) and exp sums (ACT path);
            # separate pools so their writes never false-serialize
            BM = bmp.tile([128, MT * NKC], F32, tag="bm")
            nc.vector.memset(BM, -1e30)
            AC = accp.tile([128, MT * NKC], F32, tag="ac")
            nc.vector.memset(AC, 0.0)
            ebias = persist.tile([128, 1], F32, tag="ebias")
            nc.vector.memset(ebias, -GAMMA * EXPB)
            # contiguous DoubleRow stationary layout: [cc][m][ksub 2][j 128]
            QTD = persist.tile([128, NCC * MT * 256], FP8, tag="qtd")

            def stage_load_gen(src_row0, nrt, sink, prologue=False,
                               lb=LB, sbatch=SB):
                """Generator: load+square nrt row tiles, yielding after
                each tile so the caller can interleave with sweep work.
                Appends (xt_slice, rs_ap) pairs to sink
                ([128,1] rs = 16/||row||).

                Loads: one gpsimd SWDGE casting DMA (f32 DRAM -> bf16
                SBUF) per LB row tiles.  Squares: DVE fused
                scalar_tensor_tensor x*x with accum_out (bf16 2x mode);
                prologue even tiles use ACT Square+accum instead.
                rsqrt: DVE Quake initial guess + 2 Newton steps on
                [128, SB] batches; the last Newton step folds in the
                x16 fp8 scale.
                """
                for b0 in range(0, nrt, sbatch):
                    nb = min(sbatch, nrt - b0)
                    n2c = stat.tile([128, SB], F32, tag="n2c")
                    xts = []
                    for l0 in range(0, nb, lb):
                        nl = min(lb, nb - l0)
                        row0 = src_row0 + (b0 + l0) * 128
                        xt = xpool.tile([128, LB * D], BF16, tag="xt")
                        nc.gpsimd.dma_start(
                            xt[:, 0:nl * D].rearrange(
                                "p (t d) -> p t d", t=nl),
                            X[row0:row0 + nl * 128, :].rearrange(
                                "(t p) d -> p t d", p=128),
                        )
                        for t in range(nl):
                            xs = xt[:, t * D:(t + 1) * D]
                            ti = l0 + t
                            sq = sqpool.tile([128, D], BF16, tag="sq")
                            if prologue and ti % 2 == 1:
                                nc.scalar.activation(
                                    sq, xs,
                                    mybir.ActivationFunctionType.Square,
                                    accum_out=n2c[:, ti:ti + 1])
                            else:
                                nc.vector.scalar_tensor_tensor(
                                    out=sq, in0=xs, scalar=1.0, in1=xs,
                                    op0=mybir.AluOpType.mult,
                                    op1=mybir.AluOpType.mult,
                                    accum_out=n2c[:, ti:ti + 1])
                            xts.append(xs)
                            yield
                    nn = n2c[:, 0:nb]
                    t1 = stat.tile([128, SB], U32, tag="t1")
                    nc.vector.tensor_scalar(
                        t1[:, 0:nb], nn.bitcast(U32), 1, None,
                        op0=mybir.AluOpType.logical_shift_right)
                    t2 = stat.tile([128, SB], U32, tag="t2")
                    # 0x5f3759df - t1 == ~(t1 + ~0x5f3759df); the add stays
                    # below 2^32 (the DVE uint add saturates, not wraps)
                    nc.vector.tensor_scalar(
                        t2[:, 0:nb], t1[:, 0:nb], 0xA0C8A620, None,
                        op0=mybir.AluOpType.add)
                    nc.vector.tensor_scalar(
                        t2[:, 0:nb], t2[:, 0:nb], 0xFFFFFFFF, None,
                        op0=mybir.AluOpType.bitwise_xor)
                    y0 = t2.bitcast(F32)[:, 0:nb]
                    aa = stat.tile([128, SB], F32, tag="aa")
                    nc.vector.tensor_tensor(
                        out=aa[:, 0:nb], in0=y0, in1=y0,
                        op=mybir.AluOpType.mult)
                    nc.vector.tensor_tensor(
                        out=aa[:, 0:nb], in0=aa[:, 0:nb], in1=nn,
                        op=mybir.AluOpType.mult)
                    nc.vector.tensor_scalar(
                        aa[:, 0:nb], aa[:, 0:nb], -0.5, 1.5,
                        op0=mybir.AluOpType.mult, op1=mybir.AluOpType.add)
                    y1 = stat.tile([128, SB], F32, tag="y1")
                    nc.vector.tensor_tensor(
                        out=y1[:, 0:nb], in0=y0, in1=aa[:, 0:nb],
                        op=mybir.AluOpType.mult)
                    bb = stat.tile([128, SB], F32, tag="bb")
                    nc.vector.tensor_tensor(
                        out=bb[:, 0:nb], in0=y1[:, 0:nb], in1=y1[:, 0:nb],
                        op=mybir.AluOpType.mult)
                    nc.vector.tensor_tensor(
                        out=bb[:, 0:nb], in0=bb[:, 0:nb], in1=nn,
                        op=mybir.AluOpType.mult)
                    # fold the x16 fp8 scale into the last Newton step
                    nc.vector.tensor_scalar(
                        bb[:, 0:nb], bb[:, 0:nb], -8.0, 24.0,
                        op0=mybir.AluOpType.mult, op1=mybir.AluOpType.add)
                    rsc = stat.tile([128, SB], F32, tag="rsc")
                    nc.vector.tensor_tensor(
                        out=rsc[:, 0:nb], in0=y1[:, 0:nb], in1=bb[:, 0:nb],
                        op=mybir.AluOpType.mult)
                    for t in range(nb):
                        sink.append((xts[t], rsc[:, t:t + 1]))
                    yield

            def finish_gen(pairs, dst_rows, ktc, prologue=False, ntr=1):
                """Generator: scale each staged tile into a 4-tile fp8
                batch buffer; one batched store DMA per SBB tiles (issued
                one batch late so it never waits at the SP queue head);
                then the XBAR transposes into ktc (on the ACT HWDGE to
                keep the SP queue free); yields between tiles."""
                SBB = 4
                nrows = len(pairs) * 128
                su = dst_rows.bitcast(U16)
                hr = nrows // ntr

                def xbar(h):
                    for cc in range(NCC):
                        nc.sync.dma_start_transpose(
                            ktc[:, cc * nrows + h * hr:
                                cc * nrows + (h + 1) * hr],
                            su[h * hr:(h + 1) * hr, cc * 128:(cc + 1) * 128],
                        )

                x8 = None
                pending = None  # (x8_tile, row0, nrows)
                done_rows = 0
                next_h = 0
                for t, (xs, rs) in enumerate(pairs):
                    if t % SBB == 0:
                        if pending is not None:
                            bx8, row0, nr = pending
                            nc.sync.dma_start(
                                dst_rows[row0:row0 + nr * 128, :].rearrange(
                                    "(t p) d -> p t d", p=128),
                                bx8[:, 0:nr * D].rearrange(
                                    "p (t d) -> p t d", t=nr),
                            )
                            done_rows = row0 + nr * 128
                            # fire a transpose slice as soon as its source
                            # rows are all stored
                            while (next_h < ntr - 1
                                   and (next_h + 1) * hr <= done_rows):
                                xbar(next_h)
                                next_h += 1
                        x8 = x8pool.tile([128, SBB * D], FP8, tag="x8")
                        pending = (x8, t * 128, min(SBB, len(pairs) - t))
                    if prologue:
                        eng = nc.vector if t % 2 == 1 else nc.gpsimd
                    else:
                        eng = nc.vector if t % POOL_SCALE_MOD == (
                            POOL_SCALE_MOD - 1) else nc.gpsimd
                    eng.tensor_scalar(
                        x8[:, (t % SBB) * D:(t % SBB + 1) * D], xs, rs,
                        None, op0=mybir.AluOpType.mult)
                    yield
                if pending is not None:
                    bx8, row0, nr = pending
                    nc.sync.dma_start(
                        dst_rows[row0:row0 + nr * 128, :].rearrange(
                            "(t p) d -> p t d", p=128),
                        bx8[:, 0:nr * D].rearrange("p (t d) -> p t d", t=nr),
                    )
                yield
                for h in range(next_h, ntr):
                    xbar(h)
                yield

            def run_all(gen):
                for _ in gen:
                    pass

            def sweep_chunk(kc, ktc, psA_pool, psB_pool, wa, etag,
                            feeder=None):
                kf = ktc.bitcast(FP8)  # [128, NCC*2048*2]
                nb_blocks_a = wa // 512
                for m in range(MT):
                    # two PSUM tiles per (m,kc): psA for the ACT drain,
                    # psB for the DVE drain
                    psA = psA_pool.tile([128, wa], F32, tag="psA")
                    psB = psB_pool.tile([128, 2048 - wa], F32, tag="psB")
                    # psB (the DVE exact-max tile) holds the 512-block that
                    # contains the self columns for kc=0, so the masked
                    # block is never read by the ACT exp (the mask matmul
                    # carries psB's stop, exactly the baseline structure);
                    # the remaining three blocks fill psA in order.
                    if nb_blocks_a == 3:
                        psb_set = ((m // 4),) if kc == 0 else (3,)
                    else:
                        psb_set = (2, 3)
                    tiles = []
                    pa = 0
                    pb = 0
                    for blk in range(4):
                        if blk in psb_set:
                            tiles.append((psB, pb * 512))
                            pb += 1
                        else:
                            tiles.append((psA, pa * 512))
                            pa += 1
                    # cc-outer issue order: all cc=0 matmuls first, etc.,
                    # so a chunk whose per-cc XBAR slices land staggered
                    # unblocks matmuls as early as possible (PE executes
                    # its queue in order)
                    for cc in range(NCC):
                        lhsT = QTD[
                            :, (cc * MT + m) * 256:(cc * MT + m + 1) * 256
                        ].rearrange("p (two j) -> p two j", two=2)
                        for blk in range(4):
                            ps, plo = tiles[blk]
                            needs_mask = (kc == 0 and blk == psb_set[0])
                            j0 = cc * 4096 + blk * 1024
                            rhs = kf[:, j0: j0 + 1024].rearrange(
                                "p (j two) -> p two j", two=2)
                            nc.tensor.matmul(
                                ps[:, plo:plo + 512], lhsT, rhs,
                                start=(cc == 0),
                                stop=(cc == NCC - 1 and not needs_mask),
                                perf_mode=mybir.MatmulPerfMode.DoubleRow,
                            )
                    if kc == 0:
                        # exact self-mask (chunk 0 = own rows): adds
                        # 8 * -128 * I on the self block; carries psB's
                        # stop so its reader orders after it.
                        ps, plo = tiles[psb_set[0]]
                        o = plo + (m % 4) * 128
                        nc.tensor.matmul(
                            ps[:, o:o + 128],
                            ident8,
                            masks[:, 0:128],
                            start=False, stop=True,
                            skip_group_check=True,
                        )
                    col = m * NKC + kc
                    # exact max drain of psB on DVE
                    nc.vector.reduce_max(
                        BM[:, col:col + 1], psB[:, :],
                        axis=mybir.AxisListType.X,
                    )
                    # logsumexp drain of psA on ACT:
                    # AC = sum(exp(GAMMA*(dots-EXPB)))
                    esc = escpool.tile([128, wa], BF16, tag=etag)
                    nc.scalar.activation(
                        esc, psA[:, :],
                        mybir.ActivationFunctionType.Exp,
                        scale=GAMMA, bias=ebias[:, 0:1],
                        accum_out=AC[:, col:col + 1])
                    # interleave a few staging items for future chunks so
                    # their engine work slots between this sweep's drains
                    if feeder is not None:
                        nfeed = 16 if kc == 0 else 40
                        for _ in range(nfeed):
                            if next(feeder, "done") == "done":
                                feeder = None
                                break
                if feeder is not None:
                    run_all(feeder)

            # ---- prologue: stage chunk 0 only (it doubles as the query
            # set; rows are pre-rotated per core on the host), with chunk
            # 1's loads interleaved; the first sweep needs just chunk 0 ----
            import itertools
            p0 = []
            p1 = []
            g0 = stage_load_gen(0, 16, p0, prologue=True)
            for _ in g0:
                pass
            X8C0 = dpool.tile([2048, D], FP8, tag="x8c")
            ktc0 = kpool.tile([128, NCC * 2048], U16, tag="ktc")
            for _ in itertools.zip_longest(
                finish_gen(p0, X8C0, ktc0, prologue=True),
                stage_load_gen(2048, 16, p1, prologue=True),
            ):
                pass
            # repack queries (= chunk 0) from its KTC into the contiguous
            # DoubleRow stationary layout; per-4-m groups so the first
            # sweep's stationary slices are ready before the whole repack
            for mg in range(0, MT, 4):
                for cc in range(NCC):
                    nc.vector.tensor_scalar(
                        QTD[:, (cc * MT + mg) * 256:
                            (cc * MT + mg + 4) * 256].rearrange(
                            "p (m two j) -> p m two j", m=4, two=2),
                        ktc0.bitcast(FP8)[
                            :, (cc * QPC + mg * 128) * 2:
                            (cc * QPC + (mg + 4) * 128) * 2
                        ].rearrange("p (m j two) -> p m two j", m=4, two=2),
                        1.0, None, op0=mybir.AluOpType.mult)

            # ---- staging + sweep: chunk kc sweeps while the feeder
            # interleaves finish(kc+1) then loads(kc+2) at m granularity.
            # PSUM pools are phase-split: kc 0-5 run WA=1536 with 2+2 tiles
            # (DVE is staging-loaded, so it gets the small drain); kc 6-7
            # re-pool to WA=1024 with psA x3 / psB x1 (staging done -> DVE
            # takes half the drain and 3-deep psA hides the ACT handoff) ----
            pend = {1: p1}
            ktcs = {0: ktc0}

            def make_feeder(kc):
                gens = []
                if kc + 1 < NKC:
                    X8C = dpool.tile([2048, D], FP8, tag="x8c")
                    ktcs[kc + 1] = kpool.tile(
                        [128, NCC * 2048], U16, name="ktcn", tag="ktc")
                    gens.append(finish_gen(
                        pend.pop(kc + 1), X8C, ktcs[kc + 1]))
                if kc + 2 < NKC:
                    pend[kc + 2] = []
                    gens.append(stage_load_gen(
                        (kc + 2) * 2048, 16, pend[kc + 2]))
                if len(gens) == 2:
                    # round-robin the finish and load items so scales and
                    # squares spread across the sweep
                    def _rr(a, b):
                        for x, y in itertools.zip_longest(a, b):
                            yield x
                            yield y
                    return _rr(*gens)
                if gens:
                    return itertools.chain(*gens)
                return None

            with (
                tc.tile_pool(name="psA1", bufs=2, space="PSUM") as pa1,
                tc.tile_pool(name="psB1", bufs=2, space="PSUM") as pb1,
            ):
                for kc in range(5):
                    sweep_chunk(kc, ktcs.pop(kc), pa1, pb1, 1536, "esc",
                                make_feeder(kc))
            with (
                tc.tile_pool(name="psA2", bufs=2, space="PSUM") as pa2,
                tc.tile_pool(name="psB2", bufs=2, space="PSUM") as pb2,
            ):
                for kc in range(5, NKC):
                    sweep_chunk(kc, ktcs.pop(kc), pa2, pb2, 1024, "esc2",
                                make_feeder(kc))

            # ---- finish: device only reduces per-chunk stats; the
            # scalar tail (ln, max, log-distance) runs on the host ----
            BV1 = persist.tile([128, MT], F32, tag="bv1")
            ACS = persist.tile([128, MT], F32, tag="acs")
            # two m-halves: the first half's inputs are complete ~8
            # iterations before the sweep ends, so its reduce+store can
            # overlap the tail drains
            for h0 in (0, MT // 2):
                hm = MT // 2
                nc.vector.reduce_max(
                    BV1[:, h0:h0 + hm],
                    BM[:, h0 * NKC:(h0 + hm) * NKC].rearrange(
                        "p (m k) -> p m k", k=NKC),
                    axis=mybir.AxisListType.X)
                nc.vector.reduce_sum(
                    ACS[:, h0:h0 + hm],
                    AC[:, h0 * NKC:(h0 + hm) * NKC].rearrange(
                        "p (m k) -> p m k", k=NKC),
                    axis=mybir.AxisListType.X)
                nc.sync.dma_start(BV[:, h0:h0 + hm], BV1[:, h0:h0 + hm])
                nc.sync.dma_start(AS[:, h0:h0 + hm], ACS[:, h0:h0 + hm])
    nc.compile()
    return nc


_CACHED = {}


def kernel(X: np.ndarray) -> np.ndarray:
    global LAST_EXEC_NS
    X = np.ascontiguousarray(np.asarray(X, dtype=np.float32))
    assert X.shape == (N, D)

    if "nc" not in _CACHED:
        _CACHED["nc"] = _build_nc()
    nc = _CACHED["nc"]

    eye8 = (np.eye(128) * 8.0).astype(ml_dtypes.float8_e4m3)
    # self-mask sits at chunk 0 on every core (rows are pre-rotated)
    msk = np.zeros((NKC * 128, 128), dtype=ml_dtypes.float8_e4m3)
    msk[0:128, :] = (np.eye(128) * -128.0).astype(ml_dtypes.float8_e4m3)
    in_maps = []
    for c in range(NCORES):
        in_maps.append({
            "X": np.ascontiguousarray(np.roll(X, -c * QPC, axis=0)),
            "I8": eye8,
            "MASKS": msk,
        })

    trace = os.environ.get("KOLEO_TRACE", "0") == "1"
    res = run_bass_kernel_spmd(
        nc, in_maps, core_ids=list(range(NCORES)), trace=trace,
    )
    LAST_EXEC_NS = res.exec_time_ns

    bv1 = np.concatenate(
        [r["BV"].reshape(128, MT) for r in res.results], axis=1)
    acs = np.concatenate(
        [r["AS"].reshape(128, MT) for r in res.results], axis=1)
    bv2 = np.log(np.maximum(acs, 1e-300)) / GAMMA + EXPB
    bestv = np.maximum(bv1.astype(np.float64), bv2)
    li = np.log(2.0 - bestv / 128.0)
    loss = -np.float32(0.5 * np.mean(li))
    return np.asarray(loss, dtype=np.float32)


if __name__ == "__main__":
    Xt = np.random.randn(N, D).astype(np.float32)
    print(kernel(Xt))
